# revision 1
# baseline (speedup 1.0000x reference)
"""Trainium2 Bass kernel for nn_DQA_graph (GNN message passing, DQA attention).

Strategy (data-parallel over nodes, 8 cores):
  - Nodes padded to 50176 = 8 cores x 49 tiles x 128 rows; core c owns rows
    [c*6272, (c+1)*6272).
  - Node states live in a packed DRAM table with 768B rows [x(128f) | sb(4f) |
    sa(4f) | pad], where sa/sb are the per-head DQA score contributions
    (h @ wa.T, h @ wb.T). The neighbor gather fetches x AND sb in one row read.
  - Per-node neighbor lists are pre-sorted ascending (host), so the K=32
    gather columns of a 128-node tile are order statistics; greedy grouping
    packs columns into windows whose index span fits dma_gather's int16
    range, with the window base supplied at runtime per (core, tile, window)
    from a metadata tensor (the program is SPMD-uniform across cores).
  - The gather is k-major: gathered row (k*128 + t) lands at partition t,
    chunk k -> the xg tile is directly [node t, slot k, row] with no
    transpose.
  - Scores/softmax run on ACT+DVE entirely in [t, *] layout; the weighted
    sum is a chain of 33 scalar_tensor_tensor FMAs (per-partition scalar).
  - Between propagation steps each core packs its updated rows and an
    AllGather rebuilds the replicated table.
"""
import os
import sys

sys.path.insert(0, "/opt/trn_rl_repo")
import numpy as np

import concourse.bacc as bacc
import concourse.bass as bass
import concourse.mybir as mybir
from concourse.bass_utils import run_bass_kernel_spmd
from concourse.tile import TileContext

# problem constants (hardcoded per harness contract)
N, K, S, H = 50000, 32, 128, 4
NCORES = 8
P = 128
TPC = 49                      # tiles per core
NPAD = NCORES * TPC * P       # 50176
SHARD = TPC * P               # 6272 rows per core
RW = 192                      # packed row width (f32 elements) = 768B
OFF_SB, OFF_SA = 128, 132
PACKW = 136                   # meaningful prefix of a packed row
MAXW = 32768                  # int16 index window
MAXM = 8                      # max columns per gather call (1024 idxs = SWDGE ring cap)
NEG = -50.0
ALPHA = 0.01                  # leaky relu slope
FT = mybir.dt.float32


# ----------------------------------------------------------------- host prep
def _prep_graph(neighbors, mask):
    """Sort each node's neighbors ascending, pad nodes, compute shared
    k-splits per tile position and per-(core,tile,window) bases + idx blobs.
    Returns dict with per-core input arrays and the compile-time window plan."""
    nbr = np.asarray(neighbors, np.int64)
    msk = np.asarray(mask, bool)
    order = np.argsort(nbr, axis=1, kind="stable")
    nbr_s = np.take_along_axis(nbr, order, axis=1)
    msk_s = np.take_along_axis(msk, order, axis=1)

    nbr_p = np.tile(nbr_s[N - 1], (NPAD, 1))
    nbr_p[:N] = nbr_s
    msk_p = np.zeros((NPAD, K), bool)
    msk_p[:N] = msk_s
    maskneg = np.where(msk_p, 0.0, NEG).astype(np.float32)

    # columns per (core, tile): [NCORES, TPC, K, P]
    cols = nbr_p.reshape(NCORES, TPC, P, K).transpose(0, 1, 3, 2)
    cmin = cols.min(axis=3)  # [NCORES, TPC, K]
    cmax = cols.max(axis=3)

    # shared k-split per tile position: window [k0,k1) must satisfy every core
    plan = []  # per tile position: list of (k0, k1)
    for i in range(TPC):
        wins = []
        k0 = 0
        while k0 < K:
            lo = cmin[:, i, k0].copy()
            hi = cmax[:, i, k0].copy()
            assert (hi - lo < MAXW).all(), "single column exceeds window"
            k1 = k0 + 1
            while k1 < K and k1 - k0 < MAXM:
                nlo = np.minimum(lo, cmin[:, i, k1])
                nhi = np.maximum(hi, cmax[:, i, k1])
                if (nhi - nlo >= MAXW).any():
                    break
                lo, hi = nlo, nhi
                k1 += 1
            wins.append((k0, k1))
            k0 = k1
        plan.append(wins)

    # per-core blobs
    idx_blobs, meta_blobs = [], []
    idx_cols_total = SHARD * K // 16  # 12544
    for c in range(NCORES):
        idx_blob = np.zeros((16, idx_cols_total), np.int16)
        bases = []
        off = 0
        for i in range(TPC):
            tile_cols = cols[c, i]  # [K, P]
            for (k0, k1) in plan[i]:
                base = int(tile_cols[k0:k1].min())
                bases.append(base * RW)
                rel = (tile_cols[k0:k1] - base).astype(np.int64)  # [m, P]
                assert rel.min() >= 0 and rel.max() < MAXW
                flat = rel.reshape(-1).astype(np.int16)  # k-major
                m16 = flat.shape[0] // 16
                idx_blob[:, off:off + m16] = flat.reshape(m16, 16).T
                off += m16
        assert off == idx_cols_total
        idx_blobs.append(np.tile(idx_blob, (8, 1)))  # replicate to 128 parts
        meta_blobs.append(np.asarray(bases, np.int32).reshape(1, -1))

    return {
        "plan": plan,
        "idx_blobs": idx_blobs,
        "meta_blobs": meta_blobs,
        "maskneg": maskneg,
        "n_windows": len(meta_blobs[0][0]),
    }


# ------------------------------------------------------------- device build
def _build(steps, plan, n_windows, tpc=TPC):
    level = int(os.environ.get("DQA_DEBUG_LEVEL", 6))
    """Build the SPMD Bacc module for `steps` propagation steps."""
    nc = bacc.Bacc()
    t_x = nc.dram_tensor("x_shard", [SHARD, S], FT, kind="ExternalInput")
    t_idx = nc.dram_tensor("idx_blob", [128, SHARD * K // 16], mybir.dt.int16,
                           kind="ExternalInput")
    t_meta = nc.dram_tensor("meta", [1, n_windows], mybir.dt.int32,
                            kind="ExternalInput")
    t_mn = nc.dram_tensor("maskneg", [SHARD, K], FT, kind="ExternalInput")
    t_wab = nc.dram_tensor("wab", [S, 2 * H], FT, kind="ExternalInput")
    t_bb = nc.dram_tensor("bb", [P, H], FT, kind="ExternalInput")
    t_ident = nc.dram_tensor("ident", [P, P], FT, kind="ExternalInput")
    t_out = nc.dram_tensor("out_shard", [SHARD, S], FT, kind="ExternalOutput")

    shards = [nc.dram_tensor(f"shard{s}", [SHARD, RW], FT)
              for s in range(steps)]
    tables = [nc.dram_tensor(f"table{s}", [NPAD, RW], FT, addr_space="Shared")
              for s in range(steps)]

    pool_regs = [list(nc.alloc_registers(f"gbase{j}",
                                         engines=[mybir.EngineType.Pool]))[0]
                 for j in range(8)]

    # ---------------- phase 0: pack x -> shard0 -------------------------
    with TileContext(nc) as tc:
        with tc.tile_pool(name="p0", bufs=3) as pool, \
             tc.tile_pool(name="p0c", bufs=1) as cpool, \
             tc.tile_pool(name="ps0", bufs=2, space="PSUM") as pp:
            wab = cpool.tile([S, 2 * H], FT)
            ident = cpool.tile([P, P], FT)
            nc.sync.dma_start(out=wab[:], in_=t_wab[:])
            nc.sync.dma_start(out=ident[:], in_=t_ident[:])
            for i in range(tpc):
                xt = pool.tile([P, S], FT, tag="xt")
                nc.sync.dma_start(out=xt[:], in_=t_x[i * P:(i + 1) * P, :])
                xT_ps = pp.tile([P, S], FT, tag="xT")
                nc.tensor.transpose(out=xT_ps[:], in_=xt[:], identity=ident[:])
                xT = pool.tile([S, P], FT, tag="xTs")
                nc.vector.tensor_copy(out=xT[:], in_=xT_ps[:])
                tail_ps = pp.tile([P, 2 * H], FT, tag="tail")
                nc.tensor.matmul(out=tail_ps[:], lhsT=xT[:], rhs=wab[:],
                                 start=True, stop=True)
                packed = pool.tile([P, PACKW], FT, tag="packed")
                nc.scalar.copy(out=packed[:, :S], in_=xt[:])
                nc.vector.tensor_copy(out=packed[:, S:PACKW], in_=tail_ps[:])
                nc.sync.dma_start(out=shards[0][i * P:(i + 1) * P, :PACKW],
                                  in_=packed[:])

    # ---------------- steps ---------------------------------------------
    for s in range(steps):
        if level < 2:
            break
        # AllGather shard_s -> table_s
        with nc.Block() as block, nc.semaphore(f"ccs{s}") as cc_sem:
            @block.gpsimd
            def _(gpsimd, s=s, cc_sem=cc_sem):
                gpsimd.collective_compute(
                    "AllGather", mybir.AluOpType.bypass,
                    replica_groups=[list(range(NCORES))],
                    ins=[shards[s][:]], outs=[tables[s][:]],
                ).then_inc(cc_sem, 1)
                gpsimd.wait_ge(cc_sem, 1)

        last = (s == steps - 1)
        if level < 3:
            break
        with TileContext(nc) as tc:
            with tc.tile_pool(name=f"g{s}", bufs=2) as gpool, \
                 tc.tile_pool(name=f"sm{s}", bufs=3) as spool, \
                 tc.tile_pool(name=f"c{s}", bufs=1) as cpool, \
                 tc.tile_pool(name=f"ps{s}", bufs=2, space="PSUM") as pp:
                idxb = cpool.tile([128, SHARD * K // 16], mybir.dt.int16)
                meta = cpool.tile([1, n_windows], mybir.dt.int32)
                wab = cpool.tile([S, 2 * H], FT)
                bb = cpool.tile([P, H], FT)
                ident = cpool.tile([P, P], FT)
                nc.sync.dma_start(out=idxb[:], in_=t_idx[:])
                nc.sync.dma_start(out=meta[:], in_=t_meta[:])
                nc.sync.dma_start(out=wab[:], in_=t_wab[:])
                nc.sync.dma_start(out=bb[:], in_=t_bb[:])
                nc.sync.dma_start(out=ident[:], in_=t_ident[:])

                src_rows = tables[s][:]  # [NPAD, RW]
                widx = 0
                ioff = 0
                for i in range(tpc):
                    rows = slice(i * P, (i + 1) * P)
                    xg = gpool.tile([P, K * RW], FT, tag="xg")
                    xg3 = xg[:].rearrange("p (k w) -> p k w", w=RW)
                    if os.environ.get("DQA_DEBUG_NO_GATHER"):
                        nc.vector.memset(xg[:], 0.01)
                    for (k0, k1) in plan[i]:
                        m = k1 - k0
                        if os.environ.get("DQA_DEBUG_NO_GATHER"):
                            widx += 1
                            ioff += m * P // 16
                            continue
                        reg = pool_regs[widx % len(pool_regs)]
                        nc.reg_load(reg, meta[0:1, widx:widx + 1])
                        src = bass.AP(src_rows.tensor,
                                      bass.RuntimeValue(reg), src_rows.ap)
                        n_idx = m * P
                        nc.gpsimd.dma_gather(
                            out_ap=xg3[:, k0:k1, :],
                            in_ap=src,
                            idxs_ap=idxb[:, ioff:ioff + n_idx // 16],
                            num_idxs=n_idx,
                            num_idxs_reg=n_idx,
                            elem_size=RW,
                        )
                        widx += 1
                        ioff += n_idx // 16

                    own = spool.tile([P, PACKW], FT, tag="own")
                    nc.sync.dma_start(out=own[:],
                                      in_=shards[s][rows, :PACKW])
                    mn = spool.tile([P, K], FT, tag="mn")
                    nc.sync.dma_start(out=mn[:], in_=t_mn[rows, :])

                    if level < 4:
                        outz = spool.tile([P, S], FT, tag="outz")
                        nc.vector.memset(outz[:], 0.0)
                        if last:
                            nc.sync.dma_start(out=t_out[rows, :], in_=outz[:])
                        else:
                            nc.sync.dma_start(out=shards[s + 1][rows, :S], in_=outz[:])
                        continue
                    # scores
                    sa_b = spool.tile([P, H], FT, tag="sa_b")
                    nc.vector.tensor_add(out=sa_b[:],
                                         in0=own[:, OFF_SA:OFF_SA + H],
                                         in1=bb[:])
                    e_hk = spool.tile([P, H, K], FT, tag="e_hk")
                    sb_slot = xg3[:, :, OFF_SB:OFF_SB + H].rearrange(
                        "p k h -> p h k")
                    sa_b_bc = sa_b[:].rearrange(
                        "p (h o) -> p h o", o=1).to_broadcast([P, H, K])
                    nc.vector.tensor_add(out=e_hk[:], in0=sb_slot, in1=sa_b_bc)
                    nc.vector.scalar_tensor_tensor(
                        out=e_hk[:], in0=e_hk[:], scalar=ALPHA, in1=e_hk[:],
                        op0=mybir.AluOpType.mult, op1=mybir.AluOpType.max)
                    mn_b = mn[:].rearrange(
                        "p (o k) -> p o k", o=1).to_broadcast([P, H, K])
                    nc.vector.tensor_add(out=e_hk[:], in0=e_hk[:], in1=mn_b)
                    Dn = spool.tile([P, H], FT, tag="Dn")
                    for h in range(H):
                        nc.scalar.activation(
                            out=e_hk[:, h, :], in_=e_hk[:, h, :],
                            func=mybir.ActivationFunctionType.Exp,
                            accum_out=Dn[:, h:h + 1])
                    e_self = spool.tile([P, H], FT, tag="e_self")
                    nc.vector.tensor_add(out=e_self[:], in0=sa_b[:],
                                         in1=own[:, OFF_SB:OFF_SB + H])
                    nc.vector.scalar_tensor_tensor(
                        out=e_self[:], in0=e_self[:], scalar=ALPHA,
                        in1=e_self[:], op0=mybir.AluOpType.mult,
                        op1=mybir.AluOpType.max)
                    nc.scalar.activation(
                        out=e_self[:], in_=e_self[:],
                        func=mybir.ActivationFunctionType.Exp)
                    r4 = spool.tile([P, H], FT, tag="r4")
                    nc.vector.tensor_add(out=Dn[:], in0=Dn[:], in1=e_self[:])
                    nc.vector.reciprocal(out=r4[:], in_=Dn[:])
                    nc.vector.tensor_scalar_mul(out=r4[:], in0=r4[:],
                                                scalar1=1.0 / H)
                    p_kh = spool.tile([P, K, H], FT, tag="p_kh")
                    e_as_kh = e_hk[:].rearrange("p h k -> p k h")
                    r4_b = r4[:].rearrange(
                        "p (o h) -> p o h", o=1).to_broadcast([P, K, H])
                    nc.vector.tensor_mul(out=p_kh[:], in0=e_as_kh, in1=r4_b)
                    q = spool.tile([P, K], FT, tag="q")
                    nc.vector.tensor_reduce(out=q[:], in_=p_kh[:],
                                            axis=mybir.AxisListType.X,
                                            op=mybir.AluOpType.add)
                    q0 = spool.tile([P, 1], FT, tag="q0")
                    es_r = spool.tile([P, H], FT, tag="es_r")
                    nc.vector.scalar_tensor_tensor(
                        out=es_r[:], in0=e_self[:], scalar=1.0, in1=r4[:],
                        op0=mybir.AluOpType.mult, op1=mybir.AluOpType.mult,
                        accum_out=q0[:])
                    if level < 5:
                        outz = spool.tile([P, S], FT, tag="outz")
                        nc.vector.tensor_copy(out=outz[:], in_=q[:].rearrange("p (o k) -> p o k", o=1).to_broadcast([P, 4, K]).rearrange("p a k -> p (a k)"))
                        if last:
                            nc.sync.dma_start(out=t_out[rows, :], in_=outz[:])
                        else:
                            nc.sync.dma_start(out=shards[s + 1][rows, :S], in_=outz[:])
                        continue
                    # weighted sum
                    acc = spool.tile([P, S], FT, tag="acc")
                    nc.vector.tensor_scalar(
                        out=acc[:], in0=own[:, :S], scalar1=q0[:, 0:1],
                        scalar2=None, op0=mybir.AluOpType.mult)
                    for k in range(K):
                        nc.vector.scalar_tensor_tensor(
                            out=acc[:], in0=xg3[:, k, :S],
                            scalar=q[:, k:k + 1], in1=acc[:],
                            op0=mybir.AluOpType.mult, op1=mybir.AluOpType.add)

                    if last:
                        outt = spool.tile([P, S], FT, tag="outt")
                        nc.scalar.activation(
                            out=outt[:], in_=acc[:],
                            func=mybir.ActivationFunctionType.Relu)
                        nc.sync.dma_start(out=t_out[rows, :], in_=outt[:])
                    else:
                        outt = spool.tile([P, PACKW], FT, tag="outt")
                        nc.scalar.activation(
                            out=outt[:, :S], in_=acc[:],
                            func=mybir.ActivationFunctionType.Relu)
                        oT_ps = pp.tile([P, S], FT, tag="oT")
                        nc.tensor.transpose(out=oT_ps[:], in_=outt[:, :S],
                                            identity=ident[:])
                        oT = spool.tile([S, P], FT, tag="oTs")
                        nc.vector.tensor_copy(out=oT[:], in_=oT_ps[:])
                        tail_ps = pp.tile([P, 2 * H], FT, tag="tail")
                        nc.tensor.matmul(out=tail_ps[:], lhsT=oT[:],
                                         rhs=wab[:], start=True, stop=True)
                        nc.vector.tensor_copy(out=outt[:, S:PACKW],
                                              in_=tail_ps[:])
                        nc.sync.dma_start(
                            out=shards[s + 1][rows, :PACKW], in_=outt[:])

    if level < 3:
        with TileContext(nc) as tc:
            with tc.tile_pool(name="fb", bufs=1) as pool:
                z = pool.tile([P, S], FT)
                nc.vector.memset(z[:], 0.0)
                for i in range(tpc):
                    nc.sync.dma_start(out=t_out[i * P:(i + 1) * P, :], in_=z[:])
    nc.compile()
    return nc


def make_runner(nc, in_maps):
    """Build a reusable jitted runner (mirrors bass2jax.run_bass_via_pjrt
    multi-core path, without output donation) + device-resident inputs.
    Returns (run_fn, split_fn). run_fn() executes and blocks; returns raw
    jax output arrays. split_fn(outs) -> per-core dicts."""
    import jax
    from jax.sharding import Mesh, PartitionSpec
    from jax.experimental.shard_map import shard_map
    from concourse import bass2jax
    from concourse.bass2jax import _bass_exec_p, partition_id_tensor
    import concourse.mybir as mb

    bass2jax.install_neuronx_cc_hook()
    n_cores = len(in_maps)
    partition_name = nc.partition_id_tensor.name if nc.partition_id_tensor else None
    in_names, out_names, out_avals = [], [], []
    for alloc in nc.m.functions[0].allocations:
        if not isinstance(mb.MemoryLocationSet, type) or not isinstance(alloc, mb.MemoryLocationSet):
            continue
        name = alloc.memorylocations[0].name
        if alloc.kind == "ExternalInput":
            if name != partition_name:
                in_names.append(name)
        elif alloc.kind == "ExternalOutput":
            out_names.append(name)
            out_avals.append(jax.core.ShapedArray(tuple(alloc.tensor_shape),
                                                  mb.dt.np(alloc.dtype)))
    n_params = len(in_names)
    all_in_names = list(in_names)
    if partition_name is not None:
        all_in_names.append(partition_name)

    def _body(*args):
        operands = list(args)
        if partition_name is not None:
            operands.append(partition_id_tensor())
        outs = _bass_exec_p.bind(
            *operands,
            out_avals=tuple(out_avals),
            in_names=tuple(all_in_names),
            out_names=tuple(out_names),
            lowering_input_output_aliases=(),
            sim_require_finite=True,
            sim_require_nnan=True,
            nc=nc,
        )
        return tuple(outs)

    devices = jax.devices()[:n_cores]
    mesh = Mesh(np.asarray(devices), ("core",))
    sharded = jax.jit(shard_map(_body, mesh=mesh,
                                in_specs=(PartitionSpec("core"),) * n_params,
                                out_specs=(PartitionSpec("core"),) * len(out_names),
                                check_rep=False), keep_unused=True)
    concat_in = [np.concatenate([np.asarray(in_maps[c][nm])
                                 for c in range(n_cores)], axis=0)
                 for nm in in_names]
    dev_in = jax.device_put(concat_in)
    for a in dev_in:
        a.block_until_ready()

    def run_fn():
        outs = sharded(*dev_in)
        for o in outs:
            o.block_until_ready()
        return outs

    def split_fn(outs):
        res = [dict() for _ in range(n_cores)]
        for o, nm in zip(outs, out_names):
            o = np.asarray(o)
            per = o.shape[0] // n_cores
            for c in range(n_cores):
                res[c][nm] = o[c * per:(c + 1) * per]
        return res

    return run_fn, split_fn


_CACHE = {}


def _get_module(steps, plan, n_windows):
    tpc = int(os.environ.get("DQA_DEBUG_TPC", TPC))
    key = (steps, tpc)
    if key not in _CACHE:
        _CACHE[key] = _build(steps, plan, n_windows, tpc)
    return _CACHE[key]


def _make_in_maps(inputs, g):
    x = np.asarray(inputs["x"], np.float32)
    W = np.asarray(inputs["W"], np.float32)
    b = np.asarray(inputs["b"], np.float32)
    wa, wb = W[:, :S], W[:, S:]
    x_pad = np.zeros((NPAD, S), np.float32)
    x_pad[:N] = x
    wab = np.concatenate([wb.T, wa.T], axis=1).astype(np.float32)
    bb = np.tile(b, (P, 1)).astype(np.float32)
    ident = np.eye(P, dtype=np.float32)
    in_maps = []
    for c in range(NCORES):
        rows = slice(c * SHARD, (c + 1) * SHARD)
        in_maps.append({
            "x_shard": np.ascontiguousarray(x_pad[rows]),
            "idx_blob": g["idx_blobs"][c],
            "meta": g["meta_blobs"][c],
            "maskneg": np.ascontiguousarray(g["maskneg"][rows]),
            "wab": wab,
            "bb": bb,
            "ident": ident,
        })
    return in_maps


# ------------------------------------------------------------------- kernel
def kernel(x, W, b, neighbors, mask, propagate_count):
    x = np.ascontiguousarray(np.asarray(x, np.float32))
    W = np.asarray(W, np.float32)
    b = np.asarray(b, np.float32)
    steps = int(propagate_count)
    if steps <= 0:
        return x.copy()

    wa, wb = W[:, :S], W[:, S:]
    g = _prep_graph(neighbors, mask)
    nc = _get_module(steps, g["plan"], g["n_windows"])

    in_maps = _make_in_maps({"x": x, "W": W, "b": b}, g)
    res = run_bass_kernel_spmd(nc, in_maps, list(range(NCORES)))
    out = np.concatenate([res.results[c]["out_shard"] for c in range(NCORES)],
                         axis=0)
    return np.ascontiguousarray(out[:N])


if __name__ == "__main__":
    import jax
    sys.path.insert(0, os.path.dirname(os.path.abspath(__file__)))
    import reference
    with jax.default_device(jax.devices("cpu")[0]):
        inputs = reference.setup_inputs()
        inputs = {k: (np.asarray(v) if hasattr(v, "shape") else v)
                  for k, v in inputs.items()}
        expected = np.asarray(reference.reference(**inputs))
    got = kernel(**inputs)
    rel = np.linalg.norm(got - expected) / np.linalg.norm(expected)
    print(f"Relative error: {rel:.3e}")



# revision 3
# speedup vs baseline: 2.0136x; 2.0136x over previous
"""Trainium2 Bass kernel for nn_DQA_graph (GNN message passing, DQA attention).

Strategy (data-parallel over nodes, 8 cores):
  - Nodes padded to 50176 = 8 cores x 49 tiles x 128 rows; core c owns rows
    [c*6272, (c+1)*6272).
  - Node states live in a packed DRAM table with 768B rows [x(128f) | sb(4f) |
    sa(4f) | pad], where sa/sb are the per-head DQA score contributions
    (h @ wa.T, h @ wb.T). The neighbor gather fetches x AND sb in one row read.
  - Per-node neighbor lists are pre-sorted ascending (host), so the K=32
    gather columns of a 128-node tile are order statistics; greedy grouping
    packs columns into windows whose index span fits dma_gather's int16
    range, with the window base supplied at runtime per (core, tile, window)
    from a metadata tensor (the program is SPMD-uniform across cores).
  - The gather is k-major: gathered row (k*128 + t) lands at partition t,
    chunk k -> the xg tile is directly [node t, slot k, row] with no
    transpose.
  - Scores/softmax run on ACT+DVE entirely in [t, *] layout; the weighted
    sum is a chain of 33 scalar_tensor_tensor FMAs (per-partition scalar).
  - Between propagation steps each core packs its updated rows and an
    AllGather rebuilds the replicated table.
"""
import os
import sys

sys.path.insert(0, "/opt/trn_rl_repo")
import numpy as np

import concourse.bacc as bacc
import concourse.bass as bass
import concourse.mybir as mybir
from concourse.bass_utils import run_bass_kernel_spmd
from concourse.tile import TileContext

# problem constants (hardcoded per harness contract)
N, K, S, H = 50000, 32, 128, 4
NCORES = 8
P = 128
TPC = 49                      # tiles per core
NPAD = NCORES * TPC * P       # 50176
SHARD = TPC * P               # 6272 rows per core
RW = 192                      # packed row width (f32 elements) = 768B
OFF_SB, OFF_SA = 128, 132
PACKW = 136                   # meaningful prefix of a packed row
MAXW = 32768                  # int16 index window
MAXM = 8                      # max columns per gather call (1024 idxs = SWDGE ring cap)
NEG = -50.0
ALPHA = 0.01                  # leaky relu slope
FT = mybir.dt.float32


# ----------------------------------------------------------------- host prep
def _prep_graph(neighbors, mask):
    """Sort each node's neighbors ascending, pad nodes, compute shared
    k-splits per tile position and per-(core,tile,window) bases + idx blobs.
    Returns dict with per-core input arrays and the compile-time window plan."""
    nbr = np.asarray(neighbors, np.int64)
    msk = np.asarray(mask, bool)
    order = np.argsort(nbr, axis=1, kind="stable")
    nbr_s = np.take_along_axis(nbr, order, axis=1)
    msk_s = np.take_along_axis(msk, order, axis=1)

    nbr_p = np.tile(nbr_s[N - 1], (NPAD, 1))
    nbr_p[:N] = nbr_s
    msk_p = np.zeros((NPAD, K), bool)
    msk_p[:N] = msk_s
    maskneg = np.where(msk_p, 0.0, NEG).astype(np.float32)

    # columns per (core, tile): [NCORES, TPC, K, P]
    cols = nbr_p.reshape(NCORES, TPC, P, K).transpose(0, 1, 3, 2)
    cmin = cols.min(axis=3)  # [NCORES, TPC, K]
    cmax = cols.max(axis=3)

    # shared k-split per tile position: window [k0,k1) must satisfy every core
    plan = []  # per tile position: list of (k0, k1)
    for i in range(TPC):
        wins = []
        k0 = 0
        while k0 < K:
            lo = cmin[:, i, k0].copy()
            hi = cmax[:, i, k0].copy()
            assert (hi - lo < MAXW).all(), "single column exceeds window"
            k1 = k0 + 1
            while k1 < K and k1 - k0 < MAXM:
                nlo = np.minimum(lo, cmin[:, i, k1])
                nhi = np.maximum(hi, cmax[:, i, k1])
                if (nhi - nlo >= MAXW).any():
                    break
                lo, hi = nlo, nhi
                k1 += 1
            wins.append((k0, k1))
            k0 = k1
        plan.append(wins)

    # per-core blobs
    idx_blobs, meta_blobs = [], []
    idx_cols_total = SHARD * K // 16  # 12544
    for c in range(NCORES):
        idx_blob = np.zeros((16, idx_cols_total), np.int16)
        bases = []
        off = 0
        for i in range(TPC):
            tile_cols = cols[c, i]  # [K, P]
            for (k0, k1) in plan[i]:
                base = int(tile_cols[k0:k1].min())
                bases.append(base * RW)
                rel = (tile_cols[k0:k1] - base).astype(np.int64)  # [m, P]
                assert rel.min() >= 0 and rel.max() < MAXW
                flat = rel.reshape(-1).astype(np.int16)  # k-major
                m16 = flat.shape[0] // 16
                idx_blob[:, off:off + m16] = flat.reshape(m16, 16).T
                off += m16
        assert off == idx_cols_total
        idx_blobs.append(np.tile(idx_blob, (8, 1)))  # replicate to 128 parts
        meta_blobs.append(np.asarray(bases, np.int32).reshape(1, -1))

    return {
        "plan": plan,
        "idx_blobs": idx_blobs,
        "meta_blobs": meta_blobs,
        "maskneg": maskneg,
        "n_windows": len(meta_blobs[0][0]),
    }


# ------------------------------------------------------------- device build
def _build(steps, plan, n_windows, tpc=TPC):
    level = int(os.environ.get("DQA_DEBUG_LEVEL", 6))
    """Build the SPMD Bacc module for `steps` propagation steps."""
    nc = bacc.Bacc()
    t_x = nc.dram_tensor("x_shard", [SHARD, S], FT, kind="ExternalInput")
    t_idx = nc.dram_tensor("idx_blob", [128, SHARD * K // 16], mybir.dt.int16,
                           kind="ExternalInput")
    t_meta = nc.dram_tensor("meta", [1, n_windows], mybir.dt.int32,
                            kind="ExternalInput")
    t_mn = nc.dram_tensor("maskneg", [SHARD, K], FT, kind="ExternalInput")
    t_wab = nc.dram_tensor("wab", [S, 2 * H], FT, kind="ExternalInput")
    t_bb = nc.dram_tensor("bb", [P, H], FT, kind="ExternalInput")
    t_ident = nc.dram_tensor("ident", [P, P], FT, kind="ExternalInput")
    t_out = nc.dram_tensor("out_shard", [SHARD, S], FT, kind="ExternalOutput")

    shards = [nc.dram_tensor(f"shard{s}", [SHARD, RW], FT)
              for s in range(steps)]
    tables = [nc.dram_tensor(f"table{s}", [NPAD, RW], FT, addr_space="Shared")
              for s in range(steps)]

    pool_regs = [list(nc.alloc_registers(f"gbase{j}",
                                         engines=[mybir.EngineType.Pool]))[0]
                 for j in range(8)]

    # ---------------- phase 0: pack x -> shard0 -------------------------
    with TileContext(nc) as tc:
        with tc.tile_pool(name="p0", bufs=3) as pool, \
             tc.tile_pool(name="p0c", bufs=1) as cpool, \
             tc.tile_pool(name="ps0", bufs=2, space="PSUM") as pp:
            wab = cpool.tile([S, 2 * H], FT)
            ident = cpool.tile([P, P], FT)
            nc.sync.dma_start(out=wab[:], in_=t_wab[:])
            nc.sync.dma_start(out=ident[:], in_=t_ident[:])
            for i in range(tpc):
                xt = pool.tile([P, S], FT, tag="xt")
                nc.sync.dma_start(out=xt[:], in_=t_x[i * P:(i + 1) * P, :])
                xT_ps = pp.tile([P, S], FT, tag="xT")
                nc.tensor.transpose(out=xT_ps[:], in_=xt[:], identity=ident[:])
                xT = pool.tile([S, P], FT, tag="xTs")
                nc.vector.tensor_copy(out=xT[:], in_=xT_ps[:])
                tail_ps = pp.tile([P, 2 * H], FT, tag="tail")
                nc.tensor.matmul(out=tail_ps[:], lhsT=xT[:], rhs=wab[:],
                                 start=True, stop=True)
                packed = pool.tile([P, PACKW], FT, tag="packed")
                nc.scalar.copy(out=packed[:, :S], in_=xt[:])
                nc.vector.tensor_copy(out=packed[:, S:PACKW], in_=tail_ps[:])
                nc.sync.dma_start(out=shards[0][i * P:(i + 1) * P, :PACKW],
                                  in_=packed[:])

    # ---------------- steps ---------------------------------------------
    for s in range(steps):
        if level < 2:
            break
        # AllGather shard_s -> table_s
        with nc.Block() as block, nc.semaphore(f"ccs{s}") as cc_sem:
            @block.gpsimd
            def _(gpsimd, s=s, cc_sem=cc_sem):
                gpsimd.collective_compute(
                    "AllGather", mybir.AluOpType.bypass,
                    replica_groups=[list(range(NCORES))],
                    ins=[shards[s][:]], outs=[tables[s][:]],
                ).then_inc(cc_sem, 1)
                gpsimd.wait_ge(cc_sem, 1)

        last = (s == steps - 1)
        if level < 3:
            break
        with TileContext(nc) as tc:
            with tc.tile_pool(name=f"g{s}", bufs=2) as gpool, \
                 tc.tile_pool(name=f"sm{s}", bufs=3) as spool, \
                 tc.tile_pool(name=f"c{s}", bufs=1) as cpool, \
                 tc.tile_pool(name=f"ps{s}", bufs=2, space="PSUM") as pp:
                idxb = cpool.tile([128, SHARD * K // 16], mybir.dt.int16)
                meta = cpool.tile([1, n_windows], mybir.dt.int32)
                wab = cpool.tile([S, 2 * H], FT)
                bb = cpool.tile([P, H], FT)
                ident = cpool.tile([P, P], FT)
                nc.sync.dma_start(out=idxb[:], in_=t_idx[:])
                nc.sync.dma_start(out=meta[:], in_=t_meta[:])
                nc.sync.dma_start(out=wab[:], in_=t_wab[:])
                nc.sync.dma_start(out=bb[:], in_=t_bb[:])
                nc.sync.dma_start(out=ident[:], in_=t_ident[:])

                src_rows = tables[s][:]  # [NPAD, RW]
                widx = 0
                ioff = 0
                for i in range(tpc):
                    rows = slice(i * P, (i + 1) * P)
                    xg = gpool.tile([P, K * RW], FT, tag="xg")
                    xg3 = xg[:].rearrange("p (k w) -> p k w", w=RW)
                    if os.environ.get("DQA_DEBUG_NO_GATHER"):
                        nc.vector.memset(xg[:], 0.01)
                    for (k0, k1) in plan[i]:
                        m = k1 - k0
                        if os.environ.get("DQA_DEBUG_NO_GATHER"):
                            widx += 1
                            ioff += m * P // 16
                            continue
                        reg = pool_regs[widx % len(pool_regs)]
                        nc.reg_load(reg, meta[0:1, widx:widx + 1])
                        src = bass.AP(src_rows.tensor,
                                      bass.RuntimeValue(reg), src_rows.ap)
                        n_idx = m * P
                        nc.gpsimd.dma_gather(
                            out_ap=xg3[:, k0:k1, :],
                            in_ap=src,
                            idxs_ap=idxb[:, ioff:ioff + n_idx // 16],
                            num_idxs=n_idx,
                            num_idxs_reg=n_idx,
                            elem_size=RW,
                        )
                        widx += 1
                        ioff += n_idx // 16

                    own = spool.tile([P, PACKW], FT, tag="own")
                    nc.sync.dma_start(out=own[:],
                                      in_=shards[s][rows, :PACKW])
                    mn = spool.tile([P, K], FT, tag="mn")
                    nc.sync.dma_start(out=mn[:], in_=t_mn[rows, :])

                    if level < 4:
                        outz = spool.tile([P, S], FT, tag="outz")
                        nc.vector.memset(outz[:], 0.0)
                        if last:
                            nc.sync.dma_start(out=t_out[rows, :], in_=outz[:])
                        else:
                            nc.sync.dma_start(out=shards[s + 1][rows, :S], in_=outz[:])
                        continue
                    # scores
                    sa_b = spool.tile([P, H], FT, tag="sa_b")
                    nc.vector.tensor_add(out=sa_b[:],
                                         in0=own[:, OFF_SA:OFF_SA + H],
                                         in1=bb[:])
                    e_hk = spool.tile([P, H, K], FT, tag="e_hk")
                    sb_slot = xg3[:, :, OFF_SB:OFF_SB + H].rearrange(
                        "p k h -> p h k")
                    sa_b_bc = sa_b[:].rearrange(
                        "p (h o) -> p h o", o=1).to_broadcast([P, H, K])
                    nc.vector.tensor_add(out=e_hk[:], in0=sb_slot, in1=sa_b_bc)
                    nc.vector.scalar_tensor_tensor(
                        out=e_hk[:], in0=e_hk[:], scalar=ALPHA, in1=e_hk[:],
                        op0=mybir.AluOpType.mult, op1=mybir.AluOpType.max)
                    mn_b = mn[:].rearrange(
                        "p (o k) -> p o k", o=1).to_broadcast([P, H, K])
                    nc.vector.tensor_add(out=e_hk[:], in0=e_hk[:], in1=mn_b)
                    Dn = spool.tile([P, H], FT, tag="Dn")
                    for h in range(H):
                        nc.scalar.activation(
                            out=e_hk[:, h, :], in_=e_hk[:, h, :],
                            func=mybir.ActivationFunctionType.Exp,
                            accum_out=Dn[:, h:h + 1])
                    e_self = spool.tile([P, H], FT, tag="e_self")
                    nc.vector.tensor_add(out=e_self[:], in0=sa_b[:],
                                         in1=own[:, OFF_SB:OFF_SB + H])
                    nc.vector.scalar_tensor_tensor(
                        out=e_self[:], in0=e_self[:], scalar=ALPHA,
                        in1=e_self[:], op0=mybir.AluOpType.mult,
                        op1=mybir.AluOpType.max)
                    nc.scalar.activation(
                        out=e_self[:], in_=e_self[:],
                        func=mybir.ActivationFunctionType.Exp)
                    r4 = spool.tile([P, H], FT, tag="r4")
                    nc.vector.tensor_add(out=Dn[:], in0=Dn[:], in1=e_self[:])
                    nc.vector.reciprocal(out=r4[:], in_=Dn[:])
                    nc.vector.tensor_scalar_mul(out=r4[:], in0=r4[:],
                                                scalar1=1.0 / H)
                    p_kh = spool.tile([P, K, H], FT, tag="p_kh")
                    e_as_kh = e_hk[:].rearrange("p h k -> p k h")
                    r4_b = r4[:].rearrange(
                        "p (o h) -> p o h", o=1).to_broadcast([P, K, H])
                    nc.vector.tensor_mul(out=p_kh[:], in0=e_as_kh, in1=r4_b)
                    q = spool.tile([P, K], FT, tag="q")
                    nc.vector.tensor_reduce(out=q[:], in_=p_kh[:],
                                            axis=mybir.AxisListType.X,
                                            op=mybir.AluOpType.add)
                    q0 = spool.tile([P, 1], FT, tag="q0")
                    es_r = spool.tile([P, H], FT, tag="es_r")
                    nc.vector.scalar_tensor_tensor(
                        out=es_r[:], in0=e_self[:], scalar=1.0, in1=r4[:],
                        op0=mybir.AluOpType.mult, op1=mybir.AluOpType.mult,
                        accum_out=q0[:])
                    if level < 5:
                        outz = spool.tile([P, S], FT, tag="outz")
                        nc.vector.tensor_copy(out=outz[:], in_=q[:].rearrange("p (o k) -> p o k", o=1).to_broadcast([P, 4, K]).rearrange("p a k -> p (a k)"))
                        if last:
                            nc.sync.dma_start(out=t_out[rows, :], in_=outz[:])
                        else:
                            nc.sync.dma_start(out=shards[s + 1][rows, :S], in_=outz[:])
                        continue
                    # weighted sum
                    acc = spool.tile([P, S], FT, tag="acc")
                    nc.vector.tensor_scalar(
                        out=acc[:], in0=own[:, :S], scalar1=q0[:, 0:1],
                        scalar2=None, op0=mybir.AluOpType.mult)
                    for k in range(K):
                        nc.vector.scalar_tensor_tensor(
                            out=acc[:], in0=xg3[:, k, :S],
                            scalar=q[:, k:k + 1], in1=acc[:],
                            op0=mybir.AluOpType.mult, op1=mybir.AluOpType.add)

                    if last:
                        outt = spool.tile([P, S], FT, tag="outt")
                        nc.scalar.activation(
                            out=outt[:], in_=acc[:],
                            func=mybir.ActivationFunctionType.Relu)
                        nc.sync.dma_start(out=t_out[rows, :], in_=outt[:])
                    else:
                        outt = spool.tile([P, PACKW], FT, tag="outt")
                        nc.scalar.activation(
                            out=outt[:, :S], in_=acc[:],
                            func=mybir.ActivationFunctionType.Relu)
                        oT_ps = pp.tile([P, S], FT, tag="oT")
                        nc.tensor.transpose(out=oT_ps[:], in_=outt[:, :S],
                                            identity=ident[:])
                        oT = spool.tile([S, P], FT, tag="oTs")
                        nc.vector.tensor_copy(out=oT[:], in_=oT_ps[:])
                        tail_ps = pp.tile([P, 2 * H], FT, tag="tail")
                        nc.tensor.matmul(out=tail_ps[:], lhsT=oT[:],
                                         rhs=wab[:], start=True, stop=True)
                        nc.vector.tensor_copy(out=outt[:, S:PACKW],
                                              in_=tail_ps[:])
                        nc.sync.dma_start(
                            out=shards[s + 1][rows, :PACKW], in_=outt[:])

    if level < 3:
        with TileContext(nc) as tc:
            with tc.tile_pool(name="fb", bufs=1) as pool:
                z = pool.tile([P, S], FT)
                nc.vector.memset(z[:], 0.0)
                for i in range(tpc):
                    nc.sync.dma_start(out=t_out[i * P:(i + 1) * P, :], in_=z[:])
    nc.compile()
    return nc


def make_runner(nc, in_maps):
    """Build a reusable jitted runner (mirrors bass2jax.run_bass_via_pjrt
    multi-core path, without output donation) + device-resident inputs.
    Returns (run_fn, split_fn). run_fn() executes and blocks; returns raw
    jax output arrays. split_fn(outs) -> per-core dicts."""
    import jax
    from jax.sharding import Mesh, NamedSharding, PartitionSpec
    from jax.experimental.shard_map import shard_map
    from concourse import bass2jax
    from concourse.bass2jax import _bass_exec_p, partition_id_tensor
    import concourse.mybir as mb

    bass2jax.install_neuronx_cc_hook()
    n_cores = len(in_maps)
    partition_name = nc.partition_id_tensor.name if nc.partition_id_tensor else None
    in_names, out_names, out_avals = [], [], []
    for alloc in nc.m.functions[0].allocations:
        if not isinstance(mb.MemoryLocationSet, type) or not isinstance(alloc, mb.MemoryLocationSet):
            continue
        name = alloc.memorylocations[0].name
        if alloc.kind == "ExternalInput":
            if name != partition_name:
                in_names.append(name)
        elif alloc.kind == "ExternalOutput":
            out_names.append(name)
            out_avals.append(jax.core.ShapedArray(tuple(alloc.tensor_shape),
                                                  mb.dt.np(alloc.dtype)))
    n_params = len(in_names)
    all_in_names = list(in_names)
    if partition_name is not None:
        all_in_names.append(partition_name)

    def _body(*args):
        operands = list(args)
        if partition_name is not None:
            operands.append(partition_id_tensor())
        outs = _bass_exec_p.bind(
            *operands,
            out_avals=tuple(out_avals),
            in_names=tuple(all_in_names),
            out_names=tuple(out_names),
            lowering_input_output_aliases=(),
            sim_require_finite=True,
            sim_require_nnan=True,
            nc=nc,
        )
        return tuple(outs)

    devices = jax.devices()[:n_cores]
    mesh = Mesh(np.asarray(devices), ("core",))
    sharded = jax.jit(shard_map(_body, mesh=mesh,
                                in_specs=(PartitionSpec("core"),) * n_params,
                                out_specs=(PartitionSpec("core"),) * len(out_names),
                                check_rep=False), keep_unused=True)
    concat_in = [np.concatenate([np.asarray(in_maps[c][nm])
                                 for c in range(n_cores)], axis=0)
                 for nm in in_names]
    # Pre-shard inputs across the cores so each run() dispatches with zero
    # input movement (an unsharded device_put re-scatters every call).
    shard_spec = NamedSharding(mesh, PartitionSpec("core"))
    dev_in = [jax.device_put(a, shard_spec) for a in concat_in]
    for a in dev_in:
        a.block_until_ready()

    def run_fn():
        outs = sharded(*dev_in)
        for o in outs:
            o.block_until_ready()
        return outs

    def split_fn(outs):
        res = [dict() for _ in range(n_cores)]
        for o, nm in zip(outs, out_names):
            o = np.asarray(o)
            per = o.shape[0] // n_cores
            for c in range(n_cores):
                res[c][nm] = o[c * per:(c + 1) * per]
        return res

    return run_fn, split_fn


_CACHE = {}


def _get_module(steps, plan, n_windows):
    tpc = int(os.environ.get("DQA_DEBUG_TPC", TPC))
    key = (steps, tpc)
    if key not in _CACHE:
        _CACHE[key] = _build(steps, plan, n_windows, tpc)
    return _CACHE[key]


def _make_in_maps(inputs, g):
    x = np.asarray(inputs["x"], np.float32)
    W = np.asarray(inputs["W"], np.float32)
    b = np.asarray(inputs["b"], np.float32)
    wa, wb = W[:, :S], W[:, S:]
    x_pad = np.zeros((NPAD, S), np.float32)
    x_pad[:N] = x
    wab = np.concatenate([wb.T, wa.T], axis=1).astype(np.float32)
    bb = np.tile(b, (P, 1)).astype(np.float32)
    ident = np.eye(P, dtype=np.float32)
    in_maps = []
    for c in range(NCORES):
        rows = slice(c * SHARD, (c + 1) * SHARD)
        in_maps.append({
            "x_shard": np.ascontiguousarray(x_pad[rows]),
            "idx_blob": g["idx_blobs"][c],
            "meta": g["meta_blobs"][c],
            "maskneg": np.ascontiguousarray(g["maskneg"][rows]),
            "wab": wab,
            "bb": bb,
            "ident": ident,
        })
    return in_maps


# ------------------------------------------------------------------- kernel
def kernel(x, W, b, neighbors, mask, propagate_count):
    x = np.ascontiguousarray(np.asarray(x, np.float32))
    W = np.asarray(W, np.float32)
    b = np.asarray(b, np.float32)
    steps = int(propagate_count)
    if steps <= 0:
        return x.copy()

    wa, wb = W[:, :S], W[:, S:]
    g = _prep_graph(neighbors, mask)
    nc = _get_module(steps, g["plan"], g["n_windows"])

    in_maps = _make_in_maps({"x": x, "W": W, "b": b}, g)
    res = run_bass_kernel_spmd(nc, in_maps, list(range(NCORES)))
    out = np.concatenate([res.results[c]["out_shard"] for c in range(NCORES)],
                         axis=0)
    return np.ascontiguousarray(out[:N])


if __name__ == "__main__":
    import jax
    sys.path.insert(0, os.path.dirname(os.path.abspath(__file__)))
    import reference
    with jax.default_device(jax.devices("cpu")[0]):
        inputs = reference.setup_inputs()
        inputs = {k: (np.asarray(v) if hasattr(v, "shape") else v)
                  for k, v in inputs.items()}
        expected = np.asarray(reference.reference(**inputs))
    got = kernel(**inputs)
    rel = np.linalg.norm(got - expected) / np.linalg.norm(expected)
    print(f"Relative error: {rel:.3e}")



# revision 7
# speedup vs baseline: 2.3455x; 1.1648x over previous
"""Trainium2 Bass kernel for nn_DQA_graph (GNN message passing, DQA attention).

Strategy (data-parallel over nodes, 8 cores):
  - Nodes padded to 50176 = 8 cores x 49 tiles x 128 rows; core c owns rows
    [c*6272, (c+1)*6272).
  - Node states live in a packed DRAM table whose rows hold [x | sb | sa]
    where sa/sb are the per-head DQA score contributions (h @ wa.T, h @ wb.T).
    The neighbor gather fetches x AND sb in one row read.
      * f32 mode  (DQA_BF16=0): rows of 192 f32 (768B): x f32[128] | sb | sa
      * bf16 mode (DQA_BF16=1): rows of 128 f32 (512B): x bf16[128] packed in
        64 f32 slots | sb f32[4] | sa f32[4] | pad
  - Step 0 reads a HOST-precomputed replicated table (no pack phase and no
    step-0 AllGather on device); each step s>=1 gathers from an AllGather of
    the packed rows produced by step s-1.
  - Per-node neighbor lists are pre-sorted ascending (host), so the K=32
    gather columns of a 128-node tile are order statistics; greedy grouping
    packs columns into windows whose index span fits dma_gather's int16
    range, with the window base supplied at runtime per (core, tile, window)
    from a metadata tensor (the program is SPMD-uniform across cores).
  - The gather is k-major: gathered row (k*128 + t) lands at partition t,
    chunk k -> the xg tile is directly [node t, slot k, row] with no
    transpose.
  - Scores/softmax run on ACT+DVE entirely in [t, *] layout; the weighted
    sum is 4 interleaved chains of scalar_tensor_tensor FMAs (per-partition
    scalar) for DVE ILP.
"""
import os
import sys

sys.path.insert(0, "/opt/trn_rl_repo")
import numpy as np

import concourse.bacc as bacc
import concourse.bass as bass
import concourse.mybir as mybir
from concourse.bass_utils import run_bass_kernel_spmd
from concourse.tile import TileContext

# problem constants (hardcoded per harness contract)
N, K, S, H = 50000, 32, 128, 4
NCORES = 8
P = 128
TPC = 49                      # tiles per core
NPAD = NCORES * TPC * P       # 50176
SHARD = TPC * P               # 6272 rows per core
BF16 = bool(int(os.environ.get("DQA_BF16", "1")))
if BF16:
    RW = 128                  # packed row width (f32 slots) = 512B
    XW = 64                   # f32 slots holding the (bf16) x payload
else:
    RW = 192                  # 768B rows
    XW = 128
OFF_SB, OFF_SA = XW, XW + H
PACKW = XW + 2 * H            # meaningful prefix of a packed row
MAXW = 32768                  # int16 index window (rows)
MAXM = int(os.environ.get("DQA_MAXM", "8"))   # max columns per gather call
SINGLEPKT = MAXM <= 8
NEG = -50.0
ALPHA = 0.01                  # leaky relu slope
FT = mybir.dt.float32
BT = mybir.dt.bfloat16


def _to_bf16_bits(a):
    """f32 ndarray -> uint16 bf16 bits (round to nearest even)."""
    v = a.astype(np.float32).view(np.uint32)
    r = (v + 0x7FFF + ((v >> 16) & 1)) >> 16
    return r.astype(np.uint16)


def _pack_rows(x, sb, sa):
    """Pack [n,S] f32 x (+[n,H] sb, sa) into [n, RW] f32-viewed rows."""
    n = x.shape[0]
    out = np.zeros((n, RW), np.float32)
    if BF16:
        bits = _to_bf16_bits(x)                      # [n, S] uint16
        out[:, :XW] = bits.view(np.uint32).view(np.float32)
    else:
        out[:, :XW] = x
    out[:, OFF_SB:OFF_SB + H] = sb
    out[:, OFF_SA:OFF_SA + H] = sa
    return out


# ----------------------------------------------------------------- host prep
def _prep_graph(neighbors, mask):
    """Sort each node's neighbors ascending, pad nodes, compute shared
    k-splits per tile position and per-(core,tile,window) bases + idx blobs.
    Returns dict with per-core input arrays and the compile-time window plan."""
    nbr = np.asarray(neighbors, np.int64)
    msk = np.asarray(mask, bool)
    order = np.argsort(nbr, axis=1, kind="stable")
    nbr_s = np.take_along_axis(nbr, order, axis=1)
    msk_s = np.take_along_axis(msk, order, axis=1)

    nbr_p = np.tile(nbr_s[N - 1], (NPAD, 1))
    nbr_p[:N] = nbr_s
    msk_p = np.zeros((NPAD, K), bool)
    msk_p[:N] = msk_s
    maskneg = np.where(msk_p, 0.0, NEG).astype(np.float32)

    # columns per (core, tile): [NCORES, TPC, K, P]
    cols = nbr_p.reshape(NCORES, TPC, P, K).transpose(0, 1, 3, 2)
    cmin = cols.min(axis=3)  # [NCORES, TPC, K]
    cmax = cols.max(axis=3)

    # shared k-split per tile position: window [k0,k1) must satisfy every core
    plan = []  # per tile position: list of (k0, k1)
    for i in range(TPC):
        wins = []
        k0 = 0
        while k0 < K:
            lo = cmin[:, i, k0].copy()
            hi = cmax[:, i, k0].copy()
            assert (hi - lo < MAXW).all(), "single column exceeds window"
            k1 = k0 + 1
            while k1 < K and k1 - k0 < MAXM:
                nlo = np.minimum(lo, cmin[:, i, k1])
                nhi = np.maximum(hi, cmax[:, i, k1])
                if (nhi - nlo >= MAXW).any():
                    break
                lo, hi = nlo, nhi
                k1 += 1
            wins.append((k0, k1))
            k0 = k1
        plan.append(wins)

    # per-core blobs
    idx_blobs, meta_blobs = [], []
    idx_cols_total = SHARD * K // 16  # 12544
    for c in range(NCORES):
        idx_blob = np.zeros((16, idx_cols_total), np.int16)
        bases = []
        off = 0
        for i in range(TPC):
            tile_cols = cols[c, i]  # [K, P]
            for (k0, k1) in plan[i]:
                base = int(tile_cols[k0:k1].min())
                bases.append(base * RW)
                rel = (tile_cols[k0:k1] - base).astype(np.int64)  # [m, P]
                assert rel.min() >= 0 and rel.max() < MAXW
                flat = rel.reshape(-1).astype(np.int16)  # k-major
                m16 = flat.shape[0] // 16
                idx_blob[:, off:off + m16] = flat.reshape(m16, 16).T
                off += m16
        assert off == idx_cols_total
        idx_blobs.append(np.tile(idx_blob, (8, 1)))  # replicate to 128 parts
        meta_blobs.append(np.asarray(bases, np.int32).reshape(1, -1))

    return {
        "plan": plan,
        "idx_blobs": idx_blobs,
        "meta_blobs": meta_blobs,
        "maskneg": maskneg,
        "n_windows": len(meta_blobs[0][0]),
    }


# ------------------------------------------------------------- device build
def _build(steps, plan, n_windows, tpc=TPC):
    """Build the SPMD Bacc module for `steps` propagation steps."""
    nc = bacc.Bacc()
    t_tab0 = nc.dram_tensor("table0", [NPAD, RW], FT, kind="ExternalInput")
    t_own0 = nc.dram_tensor("own0", [SHARD, RW], FT, kind="ExternalInput")
    t_idx = nc.dram_tensor("idx_blob", [128, SHARD * K // 16], mybir.dt.int16,
                           kind="ExternalInput")
    t_meta = nc.dram_tensor("meta", [1, n_windows], mybir.dt.int32,
                            kind="ExternalInput")
    t_mn = nc.dram_tensor("maskneg", [SHARD, K], FT, kind="ExternalInput")
    t_wab = nc.dram_tensor("wab", [S, 2 * H], FT, kind="ExternalInput")
    t_bb = nc.dram_tensor("bb", [P, H], FT, kind="ExternalInput")
    t_ident = nc.dram_tensor("ident", [P, P], FT, kind="ExternalInput")
    t_out = nc.dram_tensor("out_shard", [SHARD, S], FT, kind="ExternalOutput")

    shards = [nc.dram_tensor(f"shard{s}", [SHARD, RW], FT)
              for s in range(1, steps)]
    tables = [nc.dram_tensor(f"table{s}", [NPAD, RW], FT, addr_space="Shared")
              for s in range(1, steps)]

    pool_regs = [list(nc.alloc_registers(f"gbase{j}",
                                         engines=[mybir.EngineType.Pool]))[0]
                 for j in range(8)]

    for s in range(steps):
        if s > 0:
            # AllGather shard_{s} -> table_{s}
            with nc.Block() as block, nc.semaphore(f"ccs{s}") as cc_sem:
                @block.gpsimd
                def _(gpsimd, s=s, cc_sem=cc_sem):
                    gpsimd.collective_compute(
                        "AllGather", mybir.AluOpType.bypass,
                        replica_groups=[list(range(NCORES))],
                        ins=[shards[s - 1][:]], outs=[tables[s - 1][:]],
                    ).then_inc(cc_sem, 1)
                    gpsimd.wait_ge(cc_sem, 1)

        last = (s == steps - 1)
        with TileContext(nc) as tc:
            with tc.tile_pool(name=f"g{s}", bufs=3) as gpool, \
                 tc.tile_pool(name=f"sm{s}", bufs=3) as spool, \
                 tc.tile_pool(name=f"c{s}", bufs=1) as cpool, \
                 tc.tile_pool(name=f"ps{s}", bufs=2, space="PSUM") as pp:
                idxb = cpool.tile([128, SHARD * K // 16], mybir.dt.int16)
                meta = cpool.tile([1, n_windows], mybir.dt.int32)
                wab = cpool.tile([S, 2 * H], FT)
                bb = cpool.tile([P, H], FT)
                ident = cpool.tile([P, P], FT)
                nc.sync.dma_start(out=idxb[:], in_=t_idx[:])
                nc.sync.dma_start(out=meta[:], in_=t_meta[:])
                nc.sync.dma_start(out=wab[:], in_=t_wab[:])
                nc.sync.dma_start(out=bb[:], in_=t_bb[:])
                nc.sync.dma_start(out=ident[:], in_=t_ident[:])
                # bulk per-step loads: own rows + mask for all tiles
                own_src = t_own0 if s == 0 else shards[s - 1]
                own_all = cpool.tile([P, tpc, PACKW], FT)
                nc.sync.dma_start(
                    out=own_all[:],
                    in_=own_src[:tpc * P, :PACKW].rearrange(
                        "(i p) w -> p i w", p=P))
                mn_all = cpool.tile([P, tpc, K], FT)
                nc.sync.dma_start(
                    out=mn_all[:],
                    in_=t_mn[:tpc * P, :].rearrange("(i p) k -> p i k", p=P))

                src_rows = (t_tab0 if s == 0 else tables[s - 1])[:]
                widx = 0
                ioff = 0
                for i in range(tpc):
                    rows = slice(i * P, (i + 1) * P)
                    xg = gpool.tile([P, K * RW], FT, tag="xg")
                    xg3 = xg[:].rearrange("p (k w) -> p k w", w=RW)
                    for (k0, k1) in plan[i]:
                        m = k1 - k0
                        reg = pool_regs[widx % len(pool_regs)]
                        nc.reg_load(reg, meta[0:1, widx:widx + 1])
                        src = bass.AP(src_rows.tensor,
                                      bass.RuntimeValue(reg), src_rows.ap)
                        n_idx = m * P
                        nc.gpsimd.dma_gather(
                            out_ap=xg3[:, k0:k1, :],
                            in_ap=src,
                            idxs_ap=idxb[:, ioff:ioff + n_idx // 16],
                            num_idxs=n_idx,
                            num_idxs_reg=n_idx,
                            elem_size=RW,
                            single_packet=SINGLEPKT,
                        )
                        widx += 1
                        ioff += n_idx // 16

                    own = own_all[:, i, :]
                    mn = mn_all[:, i, :]

                    # scores
                    sa_b = spool.tile([P, H], FT, tag="sa_b")
                    nc.vector.tensor_add(out=sa_b[:],
                                         in0=own[:, OFF_SB + H:OFF_SB + 2 * H],
                                         in1=bb[:])
                    e_hk = spool.tile([P, H, K], FT, tag="e_hk")
                    sb_slot = xg3[:, :, OFF_SB:OFF_SB + H].rearrange(
                        "p k h -> p h k")
                    sa_b_bc = sa_b[:].rearrange(
                        "p (h o) -> p h o", o=1).to_broadcast([P, H, K])
                    nc.vector.tensor_add(out=e_hk[:], in0=sb_slot, in1=sa_b_bc)
                    nc.vector.scalar_tensor_tensor(
                        out=e_hk[:], in0=e_hk[:], scalar=ALPHA, in1=e_hk[:],
                        op0=mybir.AluOpType.mult, op1=mybir.AluOpType.max)
                    mn_b = mn.rearrange(
                        "p (o k) -> p o k", o=1).to_broadcast([P, H, K])
                    nc.vector.tensor_add(out=e_hk[:], in0=e_hk[:], in1=mn_b)
                    Dn = spool.tile([P, H], FT, tag="Dn")
                    for h in range(H):
                        nc.scalar.activation(
                            out=e_hk[:, h, :], in_=e_hk[:, h, :],
                            func=mybir.ActivationFunctionType.Exp,
                            accum_out=Dn[:, h:h + 1])
                    e_self = spool.tile([P, H], FT, tag="e_self")
                    nc.vector.tensor_add(out=e_self[:], in0=sa_b[:],
                                         in1=own[:, OFF_SB:OFF_SB + H])
                    nc.vector.scalar_tensor_tensor(
                        out=e_self[:], in0=e_self[:], scalar=ALPHA,
                        in1=e_self[:], op0=mybir.AluOpType.mult,
                        op1=mybir.AluOpType.max)
                    nc.scalar.activation(
                        out=e_self[:], in_=e_self[:],
                        func=mybir.ActivationFunctionType.Exp)
                    r4 = spool.tile([P, H], FT, tag="r4")
                    nc.vector.tensor_add(out=Dn[:], in0=Dn[:], in1=e_self[:])
                    nc.vector.reciprocal(out=r4[:], in_=Dn[:])
                    nc.vector.tensor_scalar_mul(out=r4[:], in0=r4[:],
                                                scalar1=1.0 / H)
                    p_kh = spool.tile([P, K, H], FT, tag="p_kh")
                    e_as_kh = e_hk[:].rearrange("p h k -> p k h")
                    r4_b = r4[:].rearrange(
                        "p (o h) -> p o h", o=1).to_broadcast([P, K, H])
                    nc.vector.tensor_mul(out=p_kh[:], in0=e_as_kh, in1=r4_b)
                    q = spool.tile([P, K], FT, tag="q")
                    nc.vector.tensor_reduce(out=q[:], in_=p_kh[:],
                                            axis=mybir.AxisListType.X,
                                            op=mybir.AluOpType.add)
                    q0 = spool.tile([P, 1], FT, tag="q0")
                    es_r = spool.tile([P, H], FT, tag="es_r")
                    nc.vector.scalar_tensor_tensor(
                        out=es_r[:], in0=e_self[:], scalar=1.0, in1=r4[:],
                        op0=mybir.AluOpType.mult, op1=mybir.AluOpType.mult,
                        accum_out=q0[:])

                    # x payload views (bf16 mode reads bf16 in0 directly)
                    if BF16:
                        own_x = own[:, :XW].bitcast(BT)
                        def xg_x(k):
                            return xg3[:, k, :XW].bitcast(BT)
                    else:
                        own_x = own[:, :XW]
                        def xg_x(k):
                            return xg3[:, k, :XW]

                    # weighted sum: 4 interleaved FMA chains (DVE ILP)
                    acc = spool.tile([P, S], FT, tag="acc")
                    accs = [acc]
                    for j in range(1, 4):
                        accs.append(spool.tile([P, S], FT, tag=f"acc{j}",
                                               name=f"acc{j}"))
                    nc.vector.tensor_scalar(
                        out=acc[:], in0=own_x, scalar1=q0[:, 0:1],
                        scalar2=None, op0=mybir.AluOpType.mult)
                    for j in range(1, 4):
                        nc.vector.tensor_scalar(
                            out=accs[j][:], in0=xg_x(j),
                            scalar1=q[:, j:j + 1], scalar2=None,
                            op0=mybir.AluOpType.mult)
                    for k in range(4, K):
                        a = accs[k % 4]
                        nc.vector.scalar_tensor_tensor(
                            out=a[:], in0=xg_x(k),
                            scalar=q[:, k:k + 1], in1=a[:],
                            op0=mybir.AluOpType.mult, op1=mybir.AluOpType.add)
                    nc.vector.scalar_tensor_tensor(
                        out=accs[1][:], in0=xg_x(0),
                        scalar=q[:, 0:1], in1=accs[1][:],
                        op0=mybir.AluOpType.mult, op1=mybir.AluOpType.add)
                    nc.vector.tensor_add(out=accs[2][:], in0=accs[2][:],
                                         in1=accs[3][:])
                    nc.vector.tensor_add(out=acc[:], in0=acc[:],
                                         in1=accs[1][:])
                    nc.vector.tensor_add(out=acc[:], in0=acc[:],
                                         in1=accs[2][:])

                    if last:
                        outt = spool.tile([P, S], FT, tag="outt")
                        nc.scalar.activation(
                            out=outt[:], in_=acc[:],
                            func=mybir.ActivationFunctionType.Relu)
                        nc.sync.dma_start(out=t_out[rows, :], in_=outt[:])
                    else:
                        outf = spool.tile([P, S], FT, tag="outf")
                        nc.scalar.activation(
                            out=outf[:], in_=acc[:],
                            func=mybir.ActivationFunctionType.Relu)
                        outt = spool.tile([P, PACKW], FT, tag="outt")
                        if BF16:
                            nc.vector.tensor_copy(
                                out=outt[:, :XW].bitcast(BT), in_=outf[:])
                        else:
                            nc.scalar.copy(out=outt[:, :XW], in_=outf[:])
                        oT_ps = pp.tile([P, S], FT, tag="oT")
                        nc.tensor.transpose(out=oT_ps[:], in_=outf[:],
                                            identity=ident[:])
                        oT = spool.tile([S, P], FT, tag="oTs")
                        nc.vector.tensor_copy(out=oT[:], in_=oT_ps[:])
                        tail_ps = pp.tile([P, 2 * H], FT, tag="tail")
                        nc.tensor.matmul(out=tail_ps[:], lhsT=oT[:],
                                         rhs=wab[:], start=True, stop=True)
                        nc.vector.tensor_copy(out=outt[:, XW:PACKW],
                                              in_=tail_ps[:])
                        nc.sync.dma_start(
                            out=shards[s][rows, :PACKW], in_=outt[:])

    nc.compile()
    return nc


def make_runner(nc, in_maps):
    """Build a reusable jitted runner (mirrors bass2jax.run_bass_via_pjrt
    multi-core path, without output donation) + device-resident inputs.
    Returns (run_fn, split_fn). run_fn() executes and blocks; returns raw
    jax output arrays. split_fn(outs) -> per-core dicts."""
    import jax
    from jax.sharding import Mesh, NamedSharding, PartitionSpec
    from jax.experimental.shard_map import shard_map
    from concourse import bass2jax
    from concourse.bass2jax import _bass_exec_p, partition_id_tensor
    import concourse.mybir as mb

    bass2jax.install_neuronx_cc_hook()
    n_cores = len(in_maps)
    partition_name = nc.partition_id_tensor.name if nc.partition_id_tensor else None
    in_names, out_names, out_avals = [], [], []
    for alloc in nc.m.functions[0].allocations:
        if not isinstance(mb.MemoryLocationSet, type) or not isinstance(alloc, mb.MemoryLocationSet):
            continue
        name = alloc.memorylocations[0].name
        if alloc.kind == "ExternalInput":
            if name != partition_name:
                in_names.append(name)
        elif alloc.kind == "ExternalOutput":
            out_names.append(name)
            out_avals.append(jax.core.ShapedArray(tuple(alloc.tensor_shape),
                                                  mb.dt.np(alloc.dtype)))
    n_params = len(in_names)
    all_in_names = list(in_names)
    if partition_name is not None:
        all_in_names.append(partition_name)

    def _body(*args):
        operands = list(args)
        if partition_name is not None:
            operands.append(partition_id_tensor())
        outs = _bass_exec_p.bind(
            *operands,
            out_avals=tuple(out_avals),
            in_names=tuple(all_in_names),
            out_names=tuple(out_names),
            lowering_input_output_aliases=(),
            sim_require_finite=True,
            sim_require_nnan=True,
            nc=nc,
        )
        return tuple(outs)

    devices = jax.devices()[:n_cores]
    mesh = Mesh(np.asarray(devices), ("core",))
    sharded = jax.jit(shard_map(_body, mesh=mesh,
                                in_specs=(PartitionSpec("core"),) * n_params,
                                out_specs=(PartitionSpec("core"),) * len(out_names),
                                check_rep=False), keep_unused=True)
    concat_in = [np.concatenate([np.asarray(in_maps[c][nm])
                                 for c in range(n_cores)], axis=0)
                 for nm in in_names]
    # Pre-shard inputs across the cores so each run() dispatches with zero
    # input movement (an unsharded device_put re-scatters every call).
    shard_spec = NamedSharding(mesh, PartitionSpec("core"))
    dev_in = [jax.device_put(a, shard_spec) for a in concat_in]
    for a in dev_in:
        a.block_until_ready()

    def run_fn():
        outs = sharded(*dev_in)
        for o in outs:
            o.block_until_ready()
        return outs

    def split_fn(outs):
        res = [dict() for _ in range(n_cores)]
        for o, nm in zip(outs, out_names):
            o = np.asarray(o)
            per = o.shape[0] // n_cores
            for c in range(n_cores):
                res[c][nm] = o[c * per:(c + 1) * per]
        return res

    return run_fn, split_fn


_CACHE = {}


def _get_module(steps, plan, n_windows):
    tpc = int(os.environ.get("DQA_DEBUG_TPC", TPC))
    key = (steps, tpc)
    if key not in _CACHE:
        _CACHE[key] = _build(steps, plan, n_windows, tpc)
    return _CACHE[key]


def _make_in_maps(inputs, g):
    x = np.asarray(inputs["x"], np.float32)
    W = np.asarray(inputs["W"], np.float32)
    b = np.asarray(inputs["b"], np.float32)
    wa, wb = W[:, :S], W[:, S:]
    x_pad = np.zeros((NPAD, S), np.float32)
    x_pad[:N] = x
    sb0 = x_pad @ wb.T
    sa0 = x_pad @ wa.T
    table0 = _pack_rows(x_pad, sb0, sa0)
    wab = np.concatenate([wb.T, wa.T], axis=1).astype(np.float32)
    bb = np.tile(b, (P, 1)).astype(np.float32)
    ident = np.eye(P, dtype=np.float32)
    in_maps = []
    for c in range(NCORES):
        rows = slice(c * SHARD, (c + 1) * SHARD)
        in_maps.append({
            "table0": table0,
            "own0": np.ascontiguousarray(table0[rows]),
            "idx_blob": g["idx_blobs"][c],
            "meta": g["meta_blobs"][c],
            "maskneg": np.ascontiguousarray(g["maskneg"][rows]),
            "wab": wab,
            "bb": bb,
            "ident": ident,
        })
    return in_maps


# ------------------------------------------------------------------- kernel
def kernel(x, W, b, neighbors, mask, propagate_count):
    x = np.ascontiguousarray(np.asarray(x, np.float32))
    W = np.asarray(W, np.float32)
    b = np.asarray(b, np.float32)
    steps = int(propagate_count)
    if steps <= 0:
        return x.copy()

    g = _prep_graph(neighbors, mask)
    nc = _get_module(steps, g["plan"], g["n_windows"])

    in_maps = _make_in_maps({"x": x, "W": W, "b": b}, g)
    res = run_bass_kernel_spmd(nc, in_maps, list(range(NCORES)))
    out = np.concatenate([res.results[c]["out_shard"] for c in range(NCORES)],
                         axis=0)
    return np.ascontiguousarray(out[:N])


if __name__ == "__main__":
    import jax
    sys.path.insert(0, os.path.dirname(os.path.abspath(__file__)))
    import reference
    with jax.default_device(jax.devices("cpu")[0]):
        inputs = reference.setup_inputs()
        inputs = {k: (np.asarray(v) if hasattr(v, "shape") else v)
                  for k, v in inputs.items()}
        expected = np.asarray(reference.reference(**inputs))
    got = kernel(**inputs)
    rel = np.linalg.norm(got - expected) / np.linalg.norm(expected)
    print(f"Relative error: {rel:.3e}")
